# revision 1
# baseline (speedup 1.0000x reference)
"""Trainium2 Bass kernel for nn_AttentionLayer (B=16, S=2048, D=512, H=64).

Data-parallel over batch: 8 NeuronCores x 2 batch items each; no collectives.

Math (per batch item b):
  qT = (Wq^T x^T + bq)/sqrt(H);  kT = Wk^T x^T + bk      [64, S] each
  w = exp(qT^T kT)  (no rowmax pass: |scores| <= ~9)
  out[h] = sum_t cbar[t] * x[t, :] @ Wv / S + bv, cbar = sum_s w[s,:]/Z[s]
  (V is never materialized.)

Structure:
  - fused projection stationary [Wq/sqrt(H) | Wk] -> A = [qT;kT] (32 MMs
    per batch); B = [kT;qT] via one PE permutation matmul per chunk.
  - scores row-tile halves [128,1024] by row-packed matmul pairs
    (tile_position (0,0)/(64,0)) streaming two column chunks at once.
  - exp split: most row tiles on ScalarE (activation Exp + accum rowsum);
    DVE_TILES tiles on VectorE via the Schraudolph bit-trick
    (i16 = s*log2e*128 + 16250.9 == bf16 bits of exp(s)); the i16 scratch
    is byte-copied into the bf16 w tile by a dtype-less DMA (DVE/compute
    ops on bitcast APs crash the HW; DMA APs are safe), rowsum via a
    tensor_scalar+accum_out pass. End-to-end error impact is negligible
    (softmax normalization cancels the per-element +-3%).
  - colsum in groups of 4 tiles (batched reciprocal), col-packed
    (0,32c) accumulation into one psum bank.
  - epilogue without DRAM round-trips: cbar transposed via 4 PE
    transposes (f32 identity), g transposed via 4 K=1 matmuls.
  - prologue: SWDGE cast HBM f32 -> SBUF xn bf16, bounce to DRAM,
    contiguous-read DMA transposes in per-half groups; batch-1 casts held
    on batch-0 transposes (transpose-mode switches trigger a global DMA
    drain); PE warmup matmuls hold HAM at full clock during the lead-in.

Known landmines (measured on HW):
  - bitcast APs on DVE tensor ops -> NRT_EXEC_UNIT_UNRECOVERABLE/INTERNAL.
  - SWDGE DRAM->DRAM cast DMA -> INTERNAL crash.
  - gpsimd tensor_scalar/scalar_tensor_tensor with accum_out fails to
    compile.
  - dma_start_transpose requires contiguous last-dim input (no stride-2
    f32-hi-half view of the f32 input).
  - interleaving SBUF->SBUF copies with DMA transposes on the Sync queue
    causes multi-us mode-switch drains.
"""

import numpy as np

B, S, D, H = 16, 2048, 512, 64
NCORES = 8
BPC = B // NCORES  # batches per core
P = 128
NT = S // P  # 16 row tiles
ND = D // P  # 4 d tiles
NC4 = S // 512  # 4 column chunks of 512

# Row tiles whose exp runs on VectorE (bit-trick); rest on ScalarE.
import os as _os

# global tile ids: b*16 + i
if _os.environ.get("K_DVE_EXP"):
    DVE_TILES = tuple(int(t) for t in _os.environ["K_DVE_EXP"].split(","))
else:
    DVE_TILES = ()  # Scalar-only exp measured fastest (156.7us);
    # the DVE bit-trick path (enable via K_DVE_EXP) costs more in DMA
    # bit-copies + 1x CACHE_REDUCE than it saves
K_PROBE = _os.environ.get("K_PROBE", "")  # "i16", "accum", "i16,accum"


# Schraudolph constants for bf16-bits exp: i16 = s * A16 + B16
A16 = float(np.log2(np.e) * 128.0)
B16 = 16250.91

N_WARMUP_MM = 80


def build_nc():
    import concourse.bacc as bacc
    import concourse.mybir as mybir
    import concourse.tile as tile
    from concourse.tile_rust import add_dep_helper

    f32 = mybir.dt.float32
    bf16 = mybir.dt.bfloat16
    i16 = mybir.dt.int16
    Exp = mybir.ActivationFunctionType.Exp
    Identity = mybir.ActivationFunctionType.Identity
    Copy = mybir.ActivationFunctionType.Copy
    X = mybir.AxisListType.X
    add = mybir.AluOpType.add
    mult = mybir.AluOpType.mult

    nc = bacc.Bacc("TRN2", target_bir_lowering=False)

    x_ext = nc.declare_dram_parameter("inputs", [BPC, S, D], f32, isOutput=False)
    wq_ext = nc.declare_dram_parameter("Wq", [D, H], f32, isOutput=False)
    bq_ext = nc.declare_dram_parameter("bq", [H], f32, isOutput=False)
    wk_ext = nc.declare_dram_parameter("Wk", [D, H], f32, isOutput=False)
    bk_ext = nc.declare_dram_parameter("bk", [H], f32, isOutput=False)
    wv_ext = nc.declare_dram_parameter("Wv", [D, H], f32, isOutput=False)
    bv_ext = nc.declare_dram_parameter("bv", [H], f32, isOutput=False)
    ident_ext = nc.declare_dram_parameter("ident128", [P, P], f32, isOutput=False)
    perm_ext = nc.declare_dram_parameter("perm64", [P, P], f32, isOutput=False)
    out_ext = nc.declare_dram_parameter("out", [BPC, H], f32, isOutput=True)

    inv_sqrt_h = 1.0 / float(np.sqrt(H))

    with tile.TileContext(nc) as tc:
        with (
            tc.tile_pool(name="singles", bufs=1) as singles,
            tc.tile_pool(name="xn", bufs=8) as xn_pool,
            tc.tile_pool(name="xT", bufs=16) as xT_pool,
            tc.tile_pool(name="qkT", bufs=4) as qkT_pool,
            tc.tile_pool(name="w", bufs=12) as w_pool,
            tc.tile_pool(name="sc16", bufs=4) as sc_pool,
            tc.tile_pool(name="zr", bufs=6) as zr_pool,
            tc.tile_pool(name="misc", bufs=4) as misc_pool,
            tc.tile_pool(name="dram", bufs=8, space="DRAM") as dram_pool,
            tc.tile_pool(name="mm", bufs=3, space="PSUM") as mm_pool,
            tc.tile_pool(name="col", bufs=1, space="PSUM") as col_pool,
            tc.tile_pool(name="epi", bufs=1, space="PSUM") as epi_pool,
        ):
            # ---- constants / weights prep (once) ----
            ident_sb = singles.tile([P, P], f32)
            nc.sync.dma_start(out=ident_sb, in_=ident_ext[:, :])
            perm_f = singles.tile([P, P], f32)
            nc.sync.dma_start(out=perm_f, in_=perm_ext[:, :])
            perm_sb = singles.tile([P, P], bf16)
            nc.vector.tensor_copy(out=perm_sb, in_=perm_f)
            one1 = singles.tile([1, 1], bf16)
            nc.vector.memset(one1, 1.0)

            wq_f = singles.tile([P, ND, H], f32)
            nc.sync.dma_start(out=wq_f, in_=wq_ext.rearrange("(j p) h -> p j h", p=P))
            wk_f = singles.tile([P, ND, H], f32)
            nc.sync.dma_start(out=wk_f, in_=wk_ext.rearrange("(j p) h -> p j h", p=P))
            wv_f = singles.tile([P, ND, H], f32)
            nc.sync.dma_start(out=wv_f, in_=wv_ext.rearrange("(j p) h -> p j h", p=P))

            # Fused projection stationary: wA = [Wq/sqrt(H) | Wk] per d-chunk
            # -> proj psum rows 0:64 = qT/sqrt(H), rows 64:128 = kT.
            wA = singles.tile([P, ND, P], bf16)
            for j in range(ND):
                nc.vector.tensor_scalar(
                    out=wA[:, j, 0:H], in0=wq_f[:, j, :],
                    scalar1=inv_sqrt_h, scalar2=None, op0=mult,
                )
                nc.vector.tensor_copy(out=wA[:, j, H:P], in_=wk_f[:, j, :])
            wv_b = singles.tile([P, ND, H], bf16)
            for j in range(ND):
                nc.vector.tensor_copy(out=wv_b[:, j, :], in_=wv_f[:, j, :])

            # biasA: rows 0:64 = bq/sqrt(H), rows 64:128 = bk
            biasA = singles.tile([P, 1], f32)
            nc.sync.dma_start(out=biasA[0:H, 0:1], in_=bq_ext[:, None])
            nc.sync.dma_start(out=biasA[H:P, 0:1], in_=bk_ext[:, None])
            nc.vector.tensor_scalar(
                out=biasA[0:H, 0:1], in0=biasA[0:H, 0:1],
                scalar1=inv_sqrt_h, scalar2=None, op0=mult,
            )
            bv_sb = singles.tile([1, H], f32)
            nc.sync.dma_start(out=bv_sb, in_=bv_ext[None, :])

            # ---- PE warmup: keep HAM at K=8/8 during the DMA lead-in ----
            warm_ps = epi_pool.tile([P, 512], f32, tag="epi", name="warm")
            for wi in range(N_WARMUP_MM):
                nc.tensor.matmul(
                    warm_ps[:, 0:P], lhsT=perm_sb, rhs=perm_sb,
                    start=True, stop=True,
                )

            zdump = singles.tile([P, S], bf16)

            # ---- per-batch prologue state ----
            xn_tiles = [[None] * 4 for _ in range(BPC)]  # [b][k] -> [P, 4, 512]
            qkTA = [None] * BPC
            qkTB = [None] * BPC
            prev_last_transpose = None

            def cast_half(b, h, xs_b, hold=False):
                """cast HBM f32 -> SBUF xn bf16, bounce full rows to DRAM.
                hold=True keeps the casts out of the pre-transpose drain
                window of the previous batch."""
                nonlocal prev_last_transpose
                for k in (2 * h, 2 * h + 1):
                    xn_bk = xn_pool.tile([P, 4, 512], bf16, tag="xn",
                                         name=f"xn{b}_{k}")
                    xv = x_ext[b, 512 * k : 512 * (k + 1), :].rearrange(
                        "(t p) d -> p t d", p=P
                    )
                    ci = nc.gpsimd.dma_start(out=xn_bk, in_=xv)
                    if prev_last_transpose is not None:
                        add_dep_helper(
                            ci.ins, prev_last_transpose,
                            reason="hold casts until prior transpose group",
                        )
                    ov = xs_b[512 * k : 512 * (k + 1), :].rearrange(
                        "(t p) d -> p t d", p=P
                    )
                    nc.sync.dma_start(out=ov, in_=xn_bk)
                    xn_tiles[b][k] = xn_bk

            def transpose_group(b, h, xs_b):
                """transpose one s-half (one DMA mode switch); xT per j."""
                nonlocal prev_last_transpose
                xTs = {}
                ti = None
                for j in range(ND):
                    xT_t = xT_pool.tile([P, 1024], bf16, tag="xT",
                                        name=f"xT{b}_{j}_{h}")
                    ti = nc.sync.dma_start_transpose(
                        out=xT_t,
                        in_=xs_b[1024 * h : 1024 * (h + 1), j * P : (j + 1) * P],
                    )
                    xTs[j] = xT_t
                prev_last_transpose = ti.ins
                return xTs

            def proj_chunks(b, cs, xTs, evac_engine):
                """projection chunks: all A matmuls first, then permute-B."""
                if qkTA[b] is None:
                    qkTA[b] = qkT_pool.tile([P, S], bf16, tag="qkT", name=f"qkTA{b}")
                    qkTB[b] = qkT_pool.tile([P, S], bf16, tag="qkT", name=f"qkTB{b}")
                A, Bt = qkTA[b], qkTB[b]
                for c in cs:
                    sl = slice(c * 512, (c + 1) * 512)
                    ssl = slice((c % 2) * 512, (c % 2) * 512 + 512)
                    pa = epi_pool.tile([P, 512], f32, tag="epi", name=f"pa{b}_{c}")
                    for j in range(ND):
                        nc.tensor.matmul(
                            pa, lhsT=wA[:, j, :], rhs=xTs[j][:, ssl],
                            start=(j == 0), stop=(j == ND - 1),
                        )
                    if evac_engine == "scalar":
                        nc.scalar.activation(
                            out=A[:, sl], in_=pa, func=Identity, bias=biasA[:, 0:1]
                        )
                    else:
                        nc.vector.tensor_scalar(
                            out=A[:, sl], in0=pa,
                            scalar1=biasA[:, 0:1], scalar2=None, op0=add,
                        )
                for c in cs:
                    sl = slice(c * 512, (c + 1) * 512)
                    pb = epi_pool.tile([P, 512], f32, tag="epi", name=f"pb{b}_{c}")
                    nc.tensor.matmul(pb, lhsT=perm_sb, rhs=A[:, sl],
                                     start=True, stop=True)
                    if evac_engine == "scalar":
                        nc.scalar.activation(out=Bt[:, sl], in_=pb, func=Copy)
                    else:
                        nc.vector.tensor_copy(out=Bt[:, sl], in_=pb)

            # ---- scores + exp for one row tile ----
            def scores_tile(b, i, wt, z_all, halves=(0, 1)):
                nonlocal zdump
                A, Bt = qkTA[b], qkTB[b]
                qsl = slice(i * P, (i + 1) * P)
                on_dve = (16 * b + i) in DVE_TILES
                for half in halves:
                    c0, c1 = 2 * half, 2 * half + 1
                    ps = mm_pool.tile([P, 1024], f32, tag="mm",
                                      name=f"ps{b}_{i}_{half}")
                    # row-packed pair: rows 0:64 stream kT (from B low) for
                    # chunk c0; rows 64:128 stream kT (from A high) for c1.
                    nc.tensor.matmul(
                        ps[:, 0:512], lhsT=A[0:H, qsl],
                        rhs=Bt[0:H, c0 * 512 : (c0 + 1) * 512],
                        start=True, stop=True, tile_position=(0, 0),
                    )
                    nc.tensor.matmul(
                        ps[:, 512:1024], lhsT=Bt[H:P, qsl],
                        rhs=A[H:P, c1 * 512 : (c1 + 1) * 512],
                        start=True, stop=True, tile_position=(H, 0),
                    )
                    hs = slice(half * 1024, (half + 1) * 1024)
                    if on_dve:
                        # bit-trick exp: affine to bf16 bit pattern as native
                        # i16, then a dtype-less DMA byte-copy into the bf16
                        # w tile (DVE ops on bitcast APs crash the HW).
                        sc = sc_pool.tile([P, 1024], i16, tag="wi16",
                                         name=f"wi{b}_{i}_{half}")
                        nc.vector.tensor_scalar(
                            out=sc, in0=ps,
                            scalar1=A16, scalar2=B16, op0=mult, op1=add,
                        )
                        nc.sync.dma_start(out=wt[:, hs], in_=sc.bitcast(bf16))
                    else:
                        nc.scalar.activation(
                            out=wt[:, hs], in_=ps, func=Exp,
                            accum_out=z_all[:, i, half : half + 1],
                        )
                        if "bitcast" in K_PROBE and i == 5 and b == 0:
                            pr3 = misc_pool.tile([P, 1024], i16, tag="pr3",
                                                 name=f"pr3_{half}")
                            nc.vector.tensor_scalar(
                                out=pr3, in0=ps,
                                scalar1=A16, scalar2=B16, op0=mult, op1=add,
                            )
                            pr4 = misc_pool.tile([P, 1024], bf16, tag="pr4",
                                                 name=f"pr4_{half}")
                            prz3 = misc_pool.tile([P, 1], f32, tag="prz3",
                                                  name=f"prz3_{half}")
                            nc.vector.tensor_scalar(
                                out=pr4, in0=pr3.bitcast(bf16),
                                scalar1=1.0, scalar2=None, op0=mult, op1=add,
                                accum_out=prz3,
                            )
                        if "i16" in K_PROBE and i == 3 and b == 0:
                            pr1 = misc_pool.tile([P, 1024], i16, tag="pr1",
                                                 name=f"pr1_{half}")
                            nc.vector.tensor_scalar(
                                out=pr1, in0=ps,
                                scalar1=A16, scalar2=B16, op0=mult, op1=add,
                            )
                        if "accum" in K_PROBE and i == 12 and b == 0:
                            pr2 = misc_pool.tile([P, 1024], bf16, tag="pr2",
                                                 name=f"pr2_{half}")
                            prz = misc_pool.tile([P, 1], f32, tag="prz",
                                                 name=f"prz_{half}")
                            nc.vector.tensor_scalar(
                                out=pr2, in0=wt[:, hs],
                                scalar1=1.0, scalar2=None, op0=mult, op1=add,
                                accum_out=prz,
                            )

                if on_dve and 1 in halves:
                    nc.vector.tensor_scalar(
                        out=zdump, in0=wt,
                        scalar1=1.0, scalar2=None, op0=mult, op1=add,
                        accum_out=z_all[:, i, 0:1],
                    )
                if "gst" in K_PROBE and b == 1 and i == 15 and 1 in halves:
                    g1o = misc_pool.tile([P, 1024], bf16, tag="g1o", name="g1o")
                    g1z = misc_pool.tile([P, 1], f32, tag="g1z", name="g1z")
                    nc.gpsimd.tensor_scalar(
                        out=g1o, in0=wt[:, 0:1024],
                        scalar1=1.0, scalar2=None, op0=mult, op1=add,
                        accum_out=g1z,
                    )
                if "gst2" in K_PROBE and b == 1 and i == 15 and 1 in halves:
                    g2o = misc_pool.tile([P, 1024], bf16, tag="g2o", name="g2o")
                    g2z = misc_pool.tile([P, 1], f32, tag="g2z", name="g2z")
                    nc.gpsimd.scalar_tensor_tensor(
                        out=g2o, in0=wt[:, 0:1024], scalar=1.0,
                        in1=wt[:, 0:1024], op0=mult,
                        op1=mybir.AluOpType.max, accum_out=g2z,
                    )

            def colsum_group(b, g, w_tiles, z_all, rz_all, rzb_all, colbank):
                i0 = 4 * g
                nc.vector.reduce_sum(
                    out=rz_all[:, i0 : i0 + 4], in_=z_all[:, i0 : i0 + 4, :], axis=X
                )
                nc.vector.reciprocal(
                    out=rz_all[:, i0 : i0 + 4], in_=rz_all[:, i0 : i0 + 4]
                )
                nc.vector.tensor_copy(
                    out=rzb_all[:, i0 : i0 + 4], in_=rz_all[:, i0 : i0 + 4]
                )
                for i in range(i0, i0 + 4):
                    for c in range(NC4):
                        nc.tensor.matmul(
                            colbank[32 * c : 32 * c + 1, :],
                            lhsT=rzb_all[:, i : i + 1],
                            rhs=w_tiles[i][:, c * 512 : (c + 1) * 512],
                            start=(i == 0), stop=(i == NT - 1),
                            tile_position=(0, 32 * c),
                        )

            def epilogue(b, colbank):
                cbar_sb = misc_pool.tile([P, 512], f32, tag="cbar", name=f"cbar{b}")
                nc.vector.tensor_copy(out=cbar_sb, in_=colbank)
                cbT_ps = epi_pool.tile([P, 512], f32, tag="epi", name=f"cbT{b}")
                for f in range(4):
                    nc.tensor.transpose(
                        out=cbT_ps[:, f * P : (f + 1) * P],
                        in_=cbar_sb[:, f * P : (f + 1) * P],
                        identity=ident_sb,
                    )
                # good columns of cbT_ps are 128*f + 32*c -> tile index 4c+f
                cbT_sb = misc_pool.tile([P, 4, 4], bf16, tag="cbT", name=f"cbT{b}")
                src = cbT_ps[:, :].rearrange("p (f c r) -> p c f r", f=4, c=4, r=32)
                nc.vector.tensor_copy(out=cbT_sb, in_=src[:, :, :, 0])
                gp = epi_pool.tile([1, 512], f32, tag="epi", name=f"gp{b}")
                for t in range(NT):
                    nc.tensor.matmul(
                        gp, lhsT=cbT_sb[:, t // 4, t % 4 : t % 4 + 1],
                        rhs=xn_tiles[b][t // 4][:, t % 4, :],
                        start=(t == 0), stop=(t == NT - 1),
                    )
                g_sb = misc_pool.tile([1, D], bf16, tag="g", name=f"g{b}")
                nc.vector.tensor_copy(out=g_sb, in_=gp)
                gT_ps = epi_pool.tile([P, ND], f32, tag="epi", name=f"gT{b}")
                for j in range(ND):
                    nc.tensor.matmul(
                        gT_ps[:, j : j + 1], lhsT=g_sb[0:1, j * P : (j + 1) * P],
                        rhs=one1, start=True, stop=True,
                    )
                gT_sb = misc_pool.tile([P, ND], bf16, tag="gT", name=f"gTs{b}")
                nc.vector.tensor_copy(out=gT_sb, in_=gT_ps)
                fp = epi_pool.tile([1, H], f32, tag="epi", name=f"fp{b}")
                for j in range(ND):
                    nc.tensor.matmul(
                        fp, lhsT=gT_sb[:, j : j + 1], rhs=wv_b[:, j, :],
                        start=(j == 0), stop=(j == ND - 1),
                    )
                o_sb = misc_pool.tile([1, H], f32, tag="o", name=f"o{b}")
                nc.vector.scalar_tensor_tensor(
                    out=o_sb, in0=fp, scalar=1.0 / float(S), in1=bv_sb,
                    op0=mult, op1=add,
                )
                nc.sync.dma_start(out=out_ext[b : b + 1, :], in_=o_sb)

            # ================= schedule =================
            xsd = {}
            for b in range(BPC):
                xsd[b] = dram_pool.tile([S, D], bf16, tag="xs", name=f"xs{b}")

            w_tiles = {0: [None] * NT, 1: [None] * NT}
            z_alls = {}
            rz_alls = {}
            rzb_alls = {}
            colbanks = {}
            for b in range(BPC):
                z_alls[b] = zr_pool.tile([P, NT, 2], f32, tag="z", name=f"z{b}")
                nc.vector.memset(z_alls[b], 0.0)
                rz_alls[b] = zr_pool.tile([P, NT], f32, tag="rz", name=f"rz{b}")
                rzb_alls[b] = zr_pool.tile([P, NT], bf16, tag="rzb", name=f"rzb{b}")

            def get_colbank(b):
                cb = col_pool.tile([P, 512], f32, tag="col", name=f"cb{b}")
                nc.vector.memset(cb, 0.0)
                colbanks[b] = cb

            # batch 0 prologue
            cast_half(0, 0, xsd[0])
            xT00 = transpose_group(0, 0, xsd[0])
            proj_chunks(0, (0, 1), xT00, "vector")
            cast_half(0, 1, xsd[0])
            xT01 = transpose_group(0, 1, xsd[0])

            get_colbank(0)
            za0 = z_alls[0]
            for i in range(8):
                w_tiles[0][i] = w_pool.tile([P, S], bf16, tag="w", name=f"w0_{i}")
                scores_tile(0, i, w_tiles[0][i], za0, halves=(0,))
            proj_chunks(0, (2, 3), xT01, "vector")
            for i in range(6):
                scores_tile(0, i, w_tiles[0][i], za0, halves=(1,))
                if i == 5:
                    colsum_group(0, 0, w_tiles[0], za0, rz_alls[0],
                                 rzb_alls[0], colbanks[0])

            cast_half(1, 0, xsd[1])
            cast_half(1, 1, xsd[1])

            for i in range(6, 8):
                scores_tile(0, i, w_tiles[0][i], za0, halves=(1,))
            for i in range(8, 12):
                w_tiles[0][i] = w_pool.tile([P, S], bf16, tag="w", name=f"w0_{i}")
                scores_tile(0, i, w_tiles[0][i], za0)
                if i == 9:
                    colsum_group(0, 1, w_tiles[0], za0, rz_alls[0],
                                 rzb_alls[0], colbanks[0])

            xT10 = transpose_group(1, 0, xsd[1])
            xT11 = transpose_group(1, 1, xsd[1])

            for i in range(12, NT):
                w_tiles[0][i] = w_pool.tile([P, S], bf16, tag="w", name=f"w0_{i}")
                scores_tile(0, i, w_tiles[0][i], za0)
                if i == 13:
                    colsum_group(0, 2, w_tiles[0], za0, rz_alls[0],
                                 rzb_alls[0], colbanks[0])

            proj_chunks(1, (0, 1), xT10, "vector")
            proj_chunks(1, (2, 3), xT11, "vector")

            colsum_group(0, 3, w_tiles[0], z_alls[0], rz_alls[0], rzb_alls[0],
                         colbanks[0])
            epilogue(0, colbanks[0])

            get_colbank(1)
            za1 = z_alls[1]
            for i in range(8):
                w_tiles[1][i] = w_pool.tile([P, S], bf16, tag="w", name=f"w1_{i}")
                scores_tile(1, i, w_tiles[1][i], za1, halves=(0,))
            for i in range(8):
                scores_tile(1, i, w_tiles[1][i], za1, halves=(1,))
                if i == 5:
                    colsum_group(1, 0, w_tiles[1], za1, rz_alls[1],
                                 rzb_alls[1], colbanks[1])
            for i in range(8, NT):
                w_tiles[1][i] = w_pool.tile([P, S], bf16, tag="w", name=f"w1_{i}")
                scores_tile(1, i, w_tiles[1][i], za1)
                if i == 9:
                    colsum_group(1, 1, w_tiles[1], za1, rz_alls[1],
                                 rzb_alls[1], colbanks[1])
                if i == 13:
                    colsum_group(1, 2, w_tiles[1], za1, rz_alls[1],
                                 rzb_alls[1], colbanks[1])
            colsum_group(1, 3, w_tiles[1], z_alls[1], rz_alls[1], rzb_alls[1],
                         colbanks[1])
            epilogue(1, colbanks[1])

    nc.finalize()
    return nc


_NC_CACHE = None


def _get_nc():
    global _NC_CACHE
    if _NC_CACHE is None:
        _NC_CACHE = build_nc()
    return _NC_CACHE


def run(inputs_map, trace=False, **spmd_kwargs):
    from concourse.bass_utils import run_bass_kernel_spmd

    x = np.ascontiguousarray(np.asarray(inputs_map["inputs"], dtype=np.float32))
    assert x.shape == (B, S, D), x.shape
    full = {
        "Wq": np.ascontiguousarray(np.asarray(inputs_map["Wq"], np.float32)),
        "bq": np.ascontiguousarray(np.asarray(inputs_map["bq"], np.float32)),
        "Wk": np.ascontiguousarray(np.asarray(inputs_map["Wk"], np.float32)),
        "bk": np.ascontiguousarray(np.asarray(inputs_map["bk"], np.float32)),
        "Wv": np.ascontiguousarray(np.asarray(inputs_map["Wv"], np.float32)),
        "bv": np.ascontiguousarray(np.asarray(inputs_map["bv"], np.float32)),
        "ident128": np.eye(P, dtype=np.float32),
        "perm64": np.roll(np.eye(P, dtype=np.float32), 64, axis=0),
    }
    in_maps = []
    for i in range(NCORES):
        m = {"inputs": np.ascontiguousarray(x[i * BPC : (i + 1) * BPC])}
        m.update(full)
        in_maps.append(m)
    nc = _get_nc()
    res = run_bass_kernel_spmd(
        nc, in_maps, core_ids=list(range(NCORES)), trace=trace, **spmd_kwargs
    )
    out = np.concatenate([np.asarray(res.results[i]["out"]) for i in range(NCORES)], 0)
    return out.astype(np.float32), res


def kernel(**inputs):
    out, _ = run(inputs, trace=False)
    return out


if __name__ == "__main__":
    rng = np.random.default_rng(0)
    ins = {
        "inputs": rng.standard_normal((B, S, D), dtype=np.float32),
        "Wq": rng.standard_normal((D, H), dtype=np.float32) / np.sqrt(D),
        "bq": np.zeros(H, np.float32),
        "Wk": rng.standard_normal((D, H), dtype=np.float32) / np.sqrt(D),
        "bk": np.zeros(H, np.float32),
        "Wv": rng.standard_normal((D, H), dtype=np.float32) / np.sqrt(D),
        "bv": np.zeros(H, np.float32),
    }
    out = kernel(**ins)
    print("out", out.shape, out[0, :4])

